# revision 4
# baseline (speedup 1.0000x reference)
"""Trainium2 Bass kernel for nn_Attention_56470230008033.

Multi-head self-attention (B=2, N=2048, C=1024, H=16 heads, D=64),
k = v = q, full qkv projection + output projection.

Sharding over 8 NeuronCores: data parallel on batch (2) x tensor
parallel on heads (4 head-groups of 4 heads).

v4: ScalarE-exp-bound pipeline (~143us of ACTIVATE is the floor).
  - host pre-transposes x and pre-casts x/weights to bf16: no PE
    transposes, no DVE weight/x casts, input DMA halved (6MB)
  - per (quarter, key-tile) unit covers all 4 heads: logits as 2
    row-paired walls (2 heads each, tile_position via base_partition),
    exp as 2x FD=1024 activations, PV as 2 column-paired walls into
    one PSUM bank each (heads at partitions 0-63 / 64-127), softmax
    denominators as ONE 4-way column-tiled ones-matmul wall (out
    partitions 0/32/64/96 of a shared bank)
  - PSUM: bp 2x[128,2,512] (4 banks, double buffer for the exp
    stream) + cp0/cp1 + dn + aux = 8 banks
  - input DMA split/balanced across sync/scalar/gpsimd rings with
    few large 3D descriptors, deadline-ordered; ScalarE issues only
    4 early DMAs so the exp stream is never interrupted
  - ~10us of dummy matmuls at boot warm the PE HAM clock gate
  - qkv/proj matmuls + v-units + output tiles run as deadline-paced
    fillers in the PE slack under the exp stream
"""

import sys

for _p in ("/opt/trn_rl_repo", "/opt/pypackages"):
    if _p not in sys.path:
        sys.path.append(_p)

import numpy as np
import ml_dtypes

B, N, C, H = 2, 2048, 1024, 16
D = C // H            # 64 head dim
NCORES = 8
HPC = 4               # heads per core
F = HPC * D           # 256 features per core
NT = N // 128         # 16 token tiles
CT = C // 128         # 8 contraction tiles

PVLAG = 6             # PV lag in key-tile units
NWARM = 48            # dummy matmuls to warm the PE clock gate

_CACHE = {}


def _build():
    from concourse import bacc, bass, mybir, tile

    F32 = mybir.dt.float32
    BF16 = mybir.dt.bfloat16
    AF = mybir.ActivationFunctionType

    nc = bacc.Bacc(
        "TRN2",
        target_bir_lowering=False,
        debug=False,
        enable_asserts=False,
        num_devices=NCORES,
    )
    xt_d = nc.dram_tensor("xt", [C, N], BF16, kind="ExternalInput")
    wqk_d = nc.dram_tensor("wqk", [C, 2 * F], BF16, kind="ExternalInput")
    wv_d = nc.dram_tensor("wv", [C, F], BF16, kind="ExternalInput")
    wp_d = nc.dram_tensor("wp", [F, C], BF16, kind="ExternalInput")
    bqk_d = nc.dram_tensor("bqk", [2 * F, 1], F32, kind="ExternalInput")
    bv_d = nc.dram_tensor("bv", [1, F], F32, kind="ExternalInput")
    y_d = nc.dram_tensor("y", [N, C], BF16, kind="ExternalOutput")

    scale = float(D) ** -0.5

    with tile.TileContext(nc) as tc:
        from contextlib import ExitStack

        with ExitStack() as ctx:
            const = ctx.enter_context(tc.tile_pool(name="const", bufs=1))
            persist = ctx.enter_context(tc.tile_pool(name="persist", bufs=1))

            warm = const.tile([128, 512], BF16, name="warm", tag="warm")
            ones1 = const.tile([128, 1], BF16, name="ones1", tag="ones1")
            scr_in = const.tile([1, 16], F32, name="scr_in", tag="scr_in")
            scr = const.tile([1, 16], F32, name="scr", tag="scr")
            bqk_sb = const.tile([128, 4, 1], F32, name="bqk_sb", tag="bqk_sb")
            bv1 = const.tile([1, F], F32, name="bv1", tag="bv1")
            bvb = const.tile([128, F], F32, name="bvb", tag="bvb")

            # x^T, c-major: [p, c, tok] (qk rhs / v lhsT), bf16 direct DMA
            xT4 = persist.tile([128, CT, N], BF16, name="xT4", tag="xT4")
            # qkT[0..1] = Q^T head-pairs, qkT[2..3] = K^T head-pairs
            qkT = [persist.tile([128, N], BF16, name=f"qkT{f}", tag=f"qkT{f}")
                   for f in range(4)]
            vsb = [persist.tile([128, F], BF16, name=f"vsb{t}", tag=f"vsb{t}")
                   for t in range(NT)]
            # O^T stacked per head pair (rows 0-63 head 2p, 64-127 head 2p+1)
            oT2 = [persist.tile([128, N], BF16, name=f"oT2{p}", tag=f"oT2{p}")
                   for p in range(2)]
            wqk = persist.tile([128, CT, 2 * F], BF16, name="wqk", tag="wqk")
            wv = persist.tile([128, CT, F], BF16, name="wv", tag="wv")
            wp2 = persist.tile([128, 2, C], BF16, name="wp2", tag="wp2")

            ptp = ctx.enter_context(tc.tile_pool(name="ptp", bufs=16))
            ysb = ctx.enter_context(tc.tile_pool(name="ysb", bufs=2))
            snr = ctx.enter_context(tc.tile_pool(name="snr", bufs=4))
            snb = ctx.enter_context(tc.tile_pool(name="snb", bufs=2))

            bpp = ctx.enter_context(
                tc.tile_pool(name="bpp", bufs=2, space=bass.MemorySpace.PSUM))
            cpp = ctx.enter_context(
                tc.tile_pool(name="cpp", bufs=2, space=bass.MemorySpace.PSUM))
            dnp = ctx.enter_context(
                tc.tile_pool(name="dnp", bufs=1, space=bass.MemorySpace.PSUM))
            aux = ctx.enter_context(
                tc.tile_pool(name="aux", bufs=1, space=bass.MemorySpace.PSUM))

            # ---------------- boot: warmup + exp table preload ----------
            nc.vector.memset(warm[:], 0.0)
            nc.vector.memset(ones1[:], 1.0)
            nc.vector.memset(scr_in[:], 0.0)
            nc.scalar.activation(scr[:], scr_in[:], AF.Exp)
            for _ in range(NWARM):
                wa = aux.tile([128, 512], F32, name="wa", tag="aux")
                nc.tensor.matmul(wa[:], warm[:, 0:128], warm[:],
                                 start=True, stop=True)

            # ---------------- DMA issue (deadline order per ring) -------
            xt = xt_d.ap().rearrange("(c p) n -> p c n", p=128)
            wqk_v = wqk_d.ap().rearrange("(c p) f -> p c f", p=128)
            wv_v = wv_d.ap().rearrange("(c p) f -> p c f", p=128)
            wp_v = wp_d.ap().rearrange("(t p) f -> p t f", p=128)
            bqk_v = bqk_d.ap().rearrange("(g p) o -> p g o", p=128)

            # scalar ring (HWDGE): tiny biases + prefix-critical wqk + ch1
            nc.scalar.dma_start(bqk_sb[:], bqk_v)
            nc.scalar.dma_start(bv1[:], bv_d.ap()[:])
            nc.scalar.dma_start(wqk[:, 0:6], wqk_v[:, 0:6])
            nc.scalar.dma_start(xT4[:, 0:4, 512:1024], xt[:, 0:4, 512:1024])
            # sync ring (HWDGE): x first token chunk (c 0-3) + later chunks
            nc.sync.dma_start(xT4[:, 0:4, 0:512], xt[:, 0:4, 0:512])
            nc.sync.dma_start(xT4[:, 0:4, 1024:1536], xt[:, 0:4, 1024:1536])
            nc.sync.dma_start(xT4[:, 0:4, 1536:2048], xt[:, 0:4, 1536:2048])
            # gpsimd ring (SWDGE): x (c 4-7), rest of wqk, wv, wp
            nc.gpsimd.dma_start(xT4[:, 4:8, 0:512], xt[:, 4:8, 0:512])
            nc.gpsimd.dma_start(wqk[:, 6:8], wqk_v[:, 6:8])
            nc.gpsimd.dma_start(xT4[:, 4:8, 512:1024], xt[:, 4:8, 512:1024])
            nc.gpsimd.dma_start(wv[:], wv_v)
            nc.gpsimd.dma_start(xT4[:, 4:8, 1024:1536], xt[:, 4:8, 1024:1536])
            nc.gpsimd.dma_start(xT4[:, 4:8, 1536:2048], xt[:, 4:8, 1536:2048])
            nc.gpsimd.dma_start(wp2[:], wp_v)
            nc.gpsimd.partition_broadcast(bvb[:], bv1[:])

            # ---------------- helper emitters ----------------
            def qk_unit(f, ch):
                # qkT[f][:, ch*512:(ch+1)*512] = (wqk_f^T @ x^T) + bias
                qp = aux.tile([128, 512], F32, name="qp", tag="aux")
                for c in (0, 4, 1, 5, 2, 6, 3, 7):
                    nc.tensor.matmul(
                        qp[:],
                        wqk[:, c, f * 128:(f + 1) * 128],
                        xT4[:, c, ch * 512:(ch + 1) * 512],
                        start=(c == 0), stop=(c == 7))
                nc.vector.tensor_scalar_add(
                    qkT[f][:, ch * 512:(ch + 1) * 512], qp[:],
                    bqk_sb[:, f, 0:1])

            def v_unit(t):
                # vsb[t] = (x^T_t)^T @ wv + bias   -> [128 tok, F]
                vp = aux.tile([128, F], F32, name="vp", tag="aux")
                for c in range(CT):
                    nc.tensor.matmul(
                        vp[:], xT4[:, c, t * 128:(t + 1) * 128], wv[:, c],
                        start=(c == 0), stop=(c == CT - 1))
                nc.vector.tensor_add(vsb[t][:], vp[:], bvb[:])

            ydma = [0]

            def yp_unit(t):
                ys = ysb.tile([128, 1024], BF16, name="ys", tag="ys")
                for ch in range(2):
                    yp = aux.tile([128, 512], F32, name="yp", tag="aux")
                    for p in range(2):
                        nc.tensor.matmul(
                            yp[:],
                            oT2[p][:, t * 128:(t + 1) * 128],
                            wp2[:, p, ch * 512:(ch + 1) * 512],
                            start=(p == 0), stop=(p == 1))
                    nc.vector.tensor_copy(ys[:, ch * 512:(ch + 1) * 512], yp[:])
                eng = nc.sync if ydma[0] % 2 == 0 else nc.gpsimd
                ydma[0] += 1
                eng.dma_start(y_d.ap()[t * 128:(t + 1) * 128, :], ys[:])

            # -------- filler queue (deadline ordered) -----
            # slot = global key-tile index 0..63; fillers popped one per
            # slot plus forced pops when a deadline is due
            fillers = []

            def defer(dl, fn, *a):
                fillers.append((dl, lambda: fn(*a)))

            # K chunks 1-3 due before their key tiles; Q chunks before
            # their quarter; v tiles before PV(t) at lag; yp after the
            # producing quarter's normalize
            defer(3, qk_unit, 2, 1)
            defer(4, qk_unit, 3, 1)
            defer(7, qk_unit, 2, 2)
            defer(8, qk_unit, 3, 2)
            defer(11, qk_unit, 2, 3)
            defer(12, qk_unit, 3, 3)
            defer(14, qk_unit, 0, 1)
            defer(15, qk_unit, 1, 1)
            defer(30, qk_unit, 0, 2)
            defer(31, qk_unit, 1, 2)
            defer(46, qk_unit, 0, 3)
            defer(47, qk_unit, 1, 3)
            for t in range(NT):
                defer(t + PVLAG - 1, v_unit, t)
            for t in range(12):
                defer(16 * (t // 4 + 1) + 5 + (t % 4), yp_unit, t)
            fillers.sort(key=lambda x: x[0])

            def emit_fillers(slot):
                popped = False
                while fillers and (fillers[0][0] <= slot or not popped):
                    if fillers[0][0] > slot and popped:
                        break
                    fillers.pop(0)[1]()
                    popped = True

            # ---------------- prefix: K/Q chunk 0 ----------------
            qk_unit(2, 0)
            qk_unit(3, 0)
            qk_unit(0, 0)
            qk_unit(1, 0)

            # ---------------- fused attention ----------------
            def pv_dn_unit(ent, cp0, cp1, dn):
                pt_a, pt_b, mt = ent
                st, sp = (mt == 0), (mt == NT - 1)
                nc.tensor.matmul(cp0[0:64, :], vsb[mt][:, 0:64], pt_a[:, 0],
                                 start=st, stop=sp)
                nc.tensor.matmul(cp0[64:128, :], vsb[mt][:, 64:128], pt_a[:, 1],
                                 start=st, stop=sp)
                nc.tensor.matmul(cp1[0:64, :], vsb[mt][:, 128:192], pt_b[:, 0],
                                 start=st, stop=sp)
                nc.tensor.matmul(cp1[64:128, :], vsb[mt][:, 192:256], pt_b[:, 1],
                                 start=st, stop=sp)
                nc.tensor.matmul(dn[0:1, :], ones1[:], pt_a[:, 0],
                                 start=st, stop=sp, tile_position=(0, 0))
                nc.tensor.matmul(dn[32:33, :], ones1[:], pt_a[:, 1],
                                 start=st, stop=sp, tile_position=(0, 32))
                nc.tensor.matmul(dn[64:65, :], ones1[:], pt_b[:, 0],
                                 start=st, stop=sp, tile_position=(0, 64))
                nc.tensor.matmul(dn[96:97, :], ones1[:], pt_b[:, 1],
                                 start=st, stop=sp, tile_position=(0, 96))

            def normalize(qs, cp0, cp1, dn):
                for h in range(HPC):
                    cp = cp0 if h < 2 else cp1
                    rb = 64 * (h % 2)
                    sr = snr.tile([1, 512], F32, name="sr", tag="sr")
                    nc.vector.reciprocal_approx_fast(
                        sr[:], dn[32 * h:32 * h + 1, :])
                    sb = snb.tile([64, 512], F32, name="sb", tag="sb")
                    nc.gpsimd.partition_broadcast(sb[:], sr[:])
                    nc.vector.tensor_mul(
                        oT2[h // 2][rb:rb + 64, qs:qs + 512],
                        cp[rb:rb + 64, :], sb[:])

            for quarter in range(4):
                qs = quarter * 512
                cp0 = cpp.tile([128, 512], F32, name="cp0", tag="cp")
                cp1 = cpp.tile([128, 512], F32, name="cp1", tag="cp")
                dn = dnp.tile([128, 512], F32, name="dn", tag="dn")
                pts = []
                for mt in range(NT):
                    slot = quarter * NT + mt
                    bp_a = bpp.tile([128, 2, 512], F32, name="bpa", tag="bp")
                    nc.tensor.matmul(
                        bp_a[:, 0], qkT[2][0:64, mt * 128:(mt + 1) * 128],
                        qkT[0][0:64, qs:qs + 512], start=True, stop=True)
                    nc.tensor.matmul(
                        bp_a[:, 1], qkT[2][64:128, mt * 128:(mt + 1) * 128],
                        qkT[0][64:128, qs:qs + 512], start=True, stop=True)
                    pt_a = ptp.tile([128, 2, 512], BF16, name="pta", tag="pt")
                    nc.scalar.activation(pt_a[:], bp_a[:], AF.Exp, scale=scale)
                    bp_b = bpp.tile([128, 2, 512], F32, name="bpb", tag="bp")
                    nc.tensor.matmul(
                        bp_b[:, 0], qkT[3][0:64, mt * 128:(mt + 1) * 128],
                        qkT[1][0:64, qs:qs + 512], start=True, stop=True)
                    nc.tensor.matmul(
                        bp_b[:, 1], qkT[3][64:128, mt * 128:(mt + 1) * 128],
                        qkT[1][64:128, qs:qs + 512], start=True, stop=True)
                    pt_b = ptp.tile([128, 2, 512], BF16, name="ptb", tag="pt")
                    nc.scalar.activation(pt_b[:], bp_b[:], AF.Exp, scale=scale)
                    pts.append((pt_a, pt_b, mt))
                    if len(pts) > PVLAG:
                        pv_dn_unit(pts.pop(0), cp0, cp1, dn)
                    emit_fillers(slot)
                while pts:
                    pv_dn_unit(pts.pop(0), cp0, cp1, dn)
                normalize(qs, cp0, cp1, dn)

            # tail
            while fillers:
                fillers.pop(0)[1]()
            for t in range(12, 16):
                yp_unit(t)

    nc.compile()
    return nc


def _get_nc():
    if "nc" not in _CACHE:
        _CACHE["nc"] = _build()
    return _CACHE["nc"]


def _in_maps(q, W_qkv, b_qkv, W_proj):
    bf16 = ml_dtypes.bfloat16
    # shared across cores: x^T per batch, per-group weight slices
    xts = [np.ascontiguousarray(np.asarray(q[b]).T).astype(bf16)
           for b in range(B)]
    wqks, wvs, wps, bqks, bvs = [], [], [], [], []
    for g in range(HPC):
        cols = slice(g * F, (g + 1) * F)
        wqks.append(np.ascontiguousarray(
            np.concatenate([W_qkv[:, cols], W_qkv[:, C:2 * C][:, cols]],
                           axis=1)).astype(bf16))
        wvs.append(np.ascontiguousarray(W_qkv[:, 2 * C:][:, cols]).astype(bf16))
        wps.append(np.ascontiguousarray(W_proj[cols, :]).astype(bf16))
        bqks.append(np.ascontiguousarray(
            np.concatenate([b_qkv[cols], b_qkv[C:2 * C][cols]])
            .reshape(2 * F, 1).astype(np.float32)))
        bvs.append(np.ascontiguousarray(
            b_qkv[2 * C:][cols].reshape(1, F).astype(np.float32)))
    maps = []
    for core in range(NCORES):
        b, g = divmod(core, HPC)
        maps.append({
            "xt": xts[b],
            "wqk": wqks[g],
            "wv": wvs[g],
            "wp": wps[g],
            "bqk": bqks[g],
            "bv": bvs[g],
        })
    return maps


def kernel(q, W_qkv, b_qkv, W_proj, b_proj):
    from concourse.bass_utils import run_bass_kernel_spmd

    q = np.asarray(q, dtype=np.float32)
    W_qkv = np.asarray(W_qkv, dtype=np.float32)
    b_qkv = np.asarray(b_qkv, dtype=np.float32)
    W_proj = np.asarray(W_proj, dtype=np.float32)
    b_proj = np.asarray(b_proj, dtype=np.float32)

    nc = _get_nc()
    res = run_bass_kernel_spmd(nc, _in_maps(q, W_qkv, b_qkv, W_proj),
                               core_ids=list(range(NCORES)))

    out = np.zeros((B, N, C), dtype=np.float32)
    for core in range(NCORES):
        out[core // HPC] += np.asarray(res.results[core]["y"], dtype=np.float32)
    out += b_proj
    return out
